# revision 12
# baseline (speedup 1.0000x reference)
"""CP-adapter multi-head attention on 8 Trainium2 NeuronCores.

Strategy (hardcoded for B=4, N=2048, D=1024, H=16, hd=64, R=r=64):

- Tensor-parallel over heads: core c owns heads (2c, 2c+1), i.e. columns
  [128c, 128c+128) of the q/k/v projections and rows [128c, 128c+128) of the
  output projection. Every core reads the full (transposed) activations.
- The CP adapter is linear (dropout p=0), so it folds into effective weights
  on device:  W_eff = W + U @ cp @ V  with  cp = einsum(CP_C, CP_attention).
- Matmuls run in float32r (TF32-like: operands rounded to 10-bit mantissa,
  fp32 accumulate).  Activations are pre-rounded on the host so the f32r
  matmuls are numerically exact given the rounded operands.
- Attention per (batch, head): scores computed transposed, ST = K^T-block
  stationary x Q^T moving -> PSUM [128 keys, 1024 q], one exp per stripe on
  ScalarE (free scale=1/8), PV accumulates [V|1] stationary over key blocks
  -> O' [65, 512] whose row 64 is the softmax denominator.  Division via
  reciprocal + gpsimd partition-broadcast.
- Output projection produces a per-core partial [8192, 1024]; the host sums
  the 8 partials (plus bias) -- the only cross-core reduction.
"""

from contextlib import ExitStack

import numpy as np

import concourse.bass as bass
import concourse.mybir as mybir
from concourse import bacc, tile
from concourse.bass_utils import run_bass_kernel_spmd
from concourse.masks import make_identity

F32 = mybir.dt.float32
F32R = mybir.dt.float32r
AF = mybir.ActivationFunctionType

B, N, D = 4, 2048, 1024
H, HD = 16, 64
R = 64
NCORES = 8
TOK = B * N            # 8192
CPB = D // NCORES      # 128 cols per core
ATT_SCALE = HD ** -0.5


def _round_f32r(x):
    """Round fp32 array to float32r (10-bit mantissa, RNE)."""
    u = np.ascontiguousarray(x, dtype=np.float32).view(np.uint32)
    u = (u + np.uint32(0xFFF) + ((u >> np.uint32(13)) & np.uint32(1))) \
        & np.uint32(0xFFFFE000)
    return u.view(np.float32)


def _build():
    nc = bacc.Bacc(None, target_bir_lowering=False, debug=False)

    # ---- external inputs (per-core views prepared on host) ----
    xqT = nc.dram_tensor("xqT", [16, 128, 4096], F32R, kind="ExternalInput")
    xkT = nc.dram_tensor("xkT", [16, 128, 4096], F32R, kind="ExternalInput")
    xvT = nc.dram_tensor("xvT", [16, 128, 4096], F32R, kind="ExternalInput")
    wq_c = nc.dram_tensor("wq_c", [D, CPB], F32, kind="ExternalInput")
    wk_c = nc.dram_tensor("wk_c", [D, CPB], F32, kind="ExternalInput")
    wv_c = nc.dram_tensor("wv_c", [D, CPB], F32, kind="ExternalInput")
    wp_c = nc.dram_tensor("wp_c", [CPB, D], F32, kind="ExternalInput")
    ut = nc.dram_tensor("ut", [R, D], F32, kind="ExternalInput")      # U^T
    utc = nc.dram_tensor("utc", [R, CPB], F32, kind="ExternalInput")  # U^T[:,rows_c]
    vfull = nc.dram_tensor("vfull", [R, D], F32, kind="ExternalInput")   # CP_V_W
    v_c = nc.dram_tensor("v_c", [R, CPB], F32, kind="ExternalInput")     # cols slice
    cpct = nc.dram_tensor("cpct", [R, R * R], F32, kind="ExternalInput")
    cpatt = nc.dram_tensor("cpatt", [R, 4], F32, kind="ExternalInput")

    out = nc.dram_tensor("out", [TOK, D], F32, kind="ExternalOutput")
    cpc_dram = nc.dram_tensor("cpc_dram", [4, R * R], F32)  # internal scratch

    xT3 = {"q": xqT, "k": xkT, "v": xvT}
    w_dram = {"q": wq_c, "k": wk_c, "v": wv_c}

    with tile.TileContext(nc) as tc:
        with ExitStack() as es:
            const = es.enter_context(tc.tile_pool(name="const", bufs=1))
            weffp = es.enter_context(tc.tile_pool(name="weff", bufs=1))
            prep_es = ExitStack()
            prep = prep_es.enter_context(tc.tile_pool(name="prep", bufs=1))
            wstage = prep_es.enter_context(tc.tile_pool(name="wstage", bufs=2))
            ps_prep = prep_es.enter_context(
                tc.tile_pool(name="ps_prep", bufs=2, space="PSUM"))
            # ---------- constants ----------
            identf = const.tile([128, 128], F32)
            make_identity(nc, identf)
            ident = const.tile([128, 128], F32R)
            nc.vector.tensor_copy(ident[:], identf[:])
            onesf = const.tile([128, 1], F32)
            nc.any.memset(onesf[:], 1.0)
            heat_w = const.tile([128, 128], mybir.dt.bfloat16)
            nc.vector.tensor_copy(heat_w[:], identf[:])

            def heater(n):
                # LDWEIGHTS-only PE activity: no outputs, no PSUM, no deps.
                # Keeps the PE_HAM activity window busy through DMA-bound or
                # ACT-bound stretches so the clock stays at 2.4 GHz.  Each
                # f32r matmul self-loads its weights, so clobbering the
                # stationary registers here is harmless.
                for _ in range(n):
                    nc.tensor.ldweights(heat_w[:])

            # ---------- prep: cp matrices ----------
            cpatt_sb = prep.tile([R, 4], F32)
            nc.sync.dma_start(cpatt_sb[:], cpatt[:])
            cpct_sb = prep.tile([R, 8, 512], F32)
            nc.sync.dma_start(cpct_sb[:],
                              cpct.rearrange("r (c f) -> r c f", f=512))
            cpc_sb = prep.tile([4, 8, 512], F32)
            for ch in range(8):
                cps = ps_prep.tile([4, 512], F32, name="cps", tag="cps")
                nc.tensor.matmul(cps[:], cpatt_sb[:], cpct_sb[:, ch, :],
                                 start=True, stop=True)
                nc.vector.tensor_copy(cpc_sb[:, ch, :], cps[:])
            nc.sync.dma_start(
                cpc_dram.rearrange("f (c x) -> f c x", x=512)[:], cpc_sb[:])
            # cpT[f] [b, a] = cpc_dram[f, b*64+a]
            cpT = prep.tile([R, 4, R], F32)
            nc.sync.dma_start(cpT[:],
                              cpc_dram.rearrange("f (b a) -> b f a", a=R)[:])

            # ---------- prep: effective qkv weights ----------
            vc_sb = prep.tile([R, CPB], F32)
            nc.sync.dma_start(vc_sb[:], v_c[:])
            ut_sb = prep.tile([R, 8, 128], F32)
            nc.sync.dma_start(ut_sb[:], ut.rearrange("r (c x) -> r c x", x=128))
            weff = {}
            for fi, t in enumerate("qkv"):
                t1ps = ps_prep.tile([R, CPB], F32, name="t1ps", tag="cps")
                nc.tensor.matmul(t1ps[:], cpT[:, fi, :], vc_sb[:],
                                 start=True, stop=True)
                t1 = prep.tile([R, CPB], F32, name=f"t1{t}")
                nc.vector.tensor_copy(t1[:], t1ps[:])
                wst = wstage.tile([128, 8, CPB], F32, name="wst", tag="wst")
                nc.sync.dma_start(
                    wst[:], w_dram[t].rearrange("(ko ki) j -> ki ko j", ki=128))
                we = weffp.tile([128, 8, CPB], F32R, name=f"weff{t}")
                weff[t] = we
                for db in range(8):
                    t2ps = ps_prep.tile([128, CPB], F32, name="t2ps", tag="cps")
                    nc.tensor.matmul(t2ps[:], ut_sb[:, db, :], t1[:],
                                     start=True, stop=True)
                    nc.vector.tensor_add(we[:, db, :], wst[:, db, :], t2ps[:])

            # ---------- prep: effective proj weight ----------
            vfull_sb = prep.tile([R, 2, 512], F32)
            nc.sync.dma_start(vfull_sb[:],
                              vfull.rearrange("r (c x) -> r c x", x=512))
            utc_sb = prep.tile([R, CPB], F32)
            nc.sync.dma_start(utc_sb[:], utc[:])
            t1p = prep.tile([R, 2, 512], F32)
            for ch in range(2):
                tps = ps_prep.tile([R, 512], F32, name="tps", tag="cps")
                nc.tensor.matmul(tps[:], cpT[:, 3, :], vfull_sb[:, ch, :],
                                 start=True, stop=True)
                nc.vector.tensor_copy(t1p[:, ch, :], tps[:])
            wpst = wstage.tile([CPB, 2, 512], F32, name="wst", tag="wst")
            nc.sync.dma_start(wpst[:], wp_c.rearrange("i (c x) -> i c x", x=512))
            weff_p = weffp.tile([CPB, 2, 512], F32R)
            for ch in range(2):
                tps = ps_prep.tile([CPB, 512], F32, name="t2pps", tag="cps")
                nc.tensor.matmul(tps[:], utc_sb[:], t1p[:, ch, :],
                                 start=True, stop=True)
                nc.vector.tensor_add(weff_p[:, ch, :], wpst[:, ch, :], tps[:])

            prep_es.close()
            xstream = es.enter_context(tc.tile_pool(name="xstream", bufs=32))
            qkvp = es.enter_context(tc.tile_pool(name="qkv", bufs=2))
            ptp = es.enter_context(tc.tile_pool(name="pt", bufs=3))
            normp = es.enter_context(tc.tile_pool(name="norm", bufs=4))
            outst = es.enter_context(tc.tile_pool(name="outst", bufs=3))
            ps_qkv = es.enter_context(
                tc.tile_pool(name="ps_qkv", bufs=2, space="PSUM"))
            ps_st = es.enter_context(
                tc.tile_pool(name="ps_st", bufs=2, space="PSUM"))
            ps_o = es.enter_context(
                tc.tile_pool(name="ps_o", bufs=2, space="PSUM"))

            # ---------- per-batch pipeline ----------
            # Unit-level software pipeline.  Attention is ACT(exp)-bound, so
            # next-batch qkv matmul groups are interleaved INTO the attention
            # instruction stream: the PE fills exp-wait gaps with projection
            # work, stays dense (HAM warm), and the X-stream DMA demand is
            # spread across the whole attention phase.
            def qkv_units(b):
                """Yield thunks; first call allocates destination tiles."""
                qkvT = {
                    "q": qkvp.tile([128, 4, 512], F32R, name="qT", tag="qT"),
                    "k": qkvp.tile([128, 4, 512], F32R, name="kT", tag="kT"),
                }
                v_b = qkvp.tile([128, 16, 130], F32R, name="v_b", tag="v_b")
                state = (qkvT, v_b)

                def ones_unit():
                    nc.vector.tensor_copy(
                        v_b[:, :, 64:65],
                        onesf[:, None, :].broadcast_to([128, 16, 1]))
                    nc.vector.tensor_copy(
                        v_b[:, :, 129:130],
                        onesf[:, None, :].broadcast_to([128, 16, 1]))

                def mm_unit(t, tb):
                    def f():
                        ps = ps_qkv.tile([128, 512], F32, name="psqkv",
                                         tag="psqkv")
                        for db in range(8):
                            xs = xstream.tile([128, 512], F32R,
                                              name="xs", tag="xs")
                            nc.sync.dma_start(
                                xs[:],
                                xT3[t][b * 4 + tb, :,
                                       db * 512:(db + 1) * 512])
                            nc.tensor.matmul(ps[:], weff[t][:, db, :], xs[:],
                                             start=(db == 0), stop=(db == 7))
                        if t == "v":
                            vt = outst.tile([128, 512], F32R, name="vt",
                                            tag="vt")
                            nc.vector.tensor_copy(vt[:], ps[:])
                            for j in range(4):
                                kb = tb * 4 + j
                                tp = ps_qkv.tile([128, 128], F32R,
                                                 name="pstr", tag="psqkv")
                                nc.tensor.transpose(
                                    tp[:], vt[:, j * 128:(j + 1) * 128],
                                    ident[:])
                                nc.vector.tensor_copy(v_b[:, kb, 0:64],
                                                      tp[:, 0:64])
                                nc.vector.tensor_copy(v_b[:, kb, 65:129],
                                                      tp[:, 64:128])
                        else:
                            nc.vector.tensor_copy(qkvT[t][:, tb, :], ps[:])
                    return f

                units = [ones_unit]
                for t in ("k", "q", "v"):
                    for tb in range(4):
                        units.append(mm_unit(t, tb))
                return state, units

            def attn_units(b, state):
                qkvT, v_b = state
                xaT = qkvp.tile([128, 4, 512], F32R, name="xaT", tag="xaT")

                def group(qq):
                    # one q-quarter (512 q), both heads: ST pairs share the
                    # PE array via row-tiling (K=64 each, row groups 0/64)
                    o_ps = [
                        ps_o.tile([65, 512], F32, name="o_ps", tag="o_ps")
                        for _ in range(2)
                    ]

                    def kb_unit(kb):
                        def f():
                            st = ps_st.tile([128, 1024], F32, name="st",
                                            tag="st")
                            ktb, ksub = kb // 4, kb % 4
                            for hh in range(2):
                                ro = hh * 64
                                nc.tensor.matmul(
                                    st[:, hh * 512:(hh + 1) * 512],
                                    qkvT["k"][ro:ro + 64, ktb,
                                              ksub * 128:(ksub + 1) * 128],
                                    qkvT["q"][ro:ro + 64, qq, :],
                                    start=True, stop=True)
                            pt = ptp.tile([128, 1024], F32R, name="pt",
                                          tag="pt")
                            nc.scalar.activation(pt[:], st[:], AF.Exp,
                                                 scale=ATT_SCALE)
                            for hh in range(2):
                                nc.tensor.matmul(
                                    o_ps[hh][:],
                                    v_b[:, kb, hh * 65:(hh + 1) * 65],
                                    pt[:, hh * 512:(hh + 1) * 512],
                                    start=(kb == 0), stop=(kb == 15))
                        return f

                    def norm_unit():
                        for hh in range(2):
                            ro = hh * 64
                            o_sb = normp.tile([65, 512], F32, name="o_sb",
                                              tag="o_sb")
                            nc.vector.tensor_copy(o_sb[:], o_ps[hh][:])
                            rec = normp.tile([1, 512], F32, name="rec",
                                             tag="rec")
                            nc.vector.reciprocal(rec[:], o_sb[64:65, :])
                            rec64 = normp.tile([64, 512], F32, name="rec64",
                                               tag="rec64")
                            nc.gpsimd.partition_broadcast(rec64[:], rec[:])
                            nc.vector.tensor_mul(xaT[ro:ro + 64, qq, :],
                                                 o_sb[0:64, :], rec64[:])
                    return [kb_unit(kb) for kb in range(16)] + [norm_unit]

                return xaT, [group(qq) for qq in range(4)]

            def proj_units(b, xaT, qq):
                tok0 = b * N

                def tb_unit(tb):
                    def f():
                        sub = tb % 4
                        lx = xaT[:, qq, sub * 128:(sub + 1) * 128]
                        ob = outst.tile([128, 1024], F32, name="ob", tag="ob")
                        for ch in range(2):
                            ps = ps_qkv.tile([128, 512], F32, name="pspj",
                                             tag="psqkv")
                            nc.tensor.matmul(ps[:], lx, weff_p[:, ch, :],
                                             start=True, stop=True)
                            nc.vector.tensor_copy(
                                ob[:, ch * 512:(ch + 1) * 512], ps[:])
                        nc.sync.dma_start(
                            out[tok0 + tb * 128:tok0 + (tb + 1) * 128, :],
                            ob[:])
                    return f
                return [tb_unit(qq * 4 + j) for j in range(4)]

            state, units0 = qkv_units(0)
            heater(16)
            for u in units0:
                u()
                heater(24)
            for b in range(B):
                xaT, groups = attn_units(b, state)
                fill = []
                if b + 1 < B:
                    state, fill = qkv_units(b + 1)
                fi = 0
                for qq, g_units in enumerate(groups):
                    for i, u in enumerate(g_units):
                        u()
                        if i % 4 == 3:
                            if fi < len(fill):
                                fill[fi]()
                                fi += 1
                            else:
                                heater(6)
                    heater(10)
                    for u in proj_units(b, xaT, qq):
                        u()
                for u in fill[fi:]:
                    u()
    nc.compile()
    return nc


_NC = None


def _get_nc():
    global _NC
    if _NC is None:
        _NC = _build()
    return _NC


def _prep_in_maps(inputs):
    f32 = lambda a: np.ascontiguousarray(np.asarray(a), dtype=np.float32)
    xq = f32(inputs["input_q"]).reshape(TOK, D)
    xk = f32(inputs["input_k"]).reshape(TOK, D)
    xv = f32(inputs["input_v"]).reshape(TOK, D)
    def tile_xt(x):
        # [TOK, D] -> [16, 128, 4096]: tile t holds tokens [512t, 512t+512),
        # laid out [ki, ko*512 + j] with d = ko*128 + ki.
        xt = _round_f32r(x.T)                       # [D, TOK]
        xt = xt.reshape(8, 128, 16, 512)            # ko ki t j
        xt = xt.transpose(2, 1, 0, 3)               # t ki ko j
        return np.ascontiguousarray(xt.reshape(16, 128, 4096))
    xqT = tile_xt(xq)
    xkT = tile_xt(xk)
    xvT = tile_xt(xv)
    Wq, Wk, Wv = f32(inputs["Wq"]), f32(inputs["Wk"]), f32(inputs["Wv"])
    Wp = f32(inputs["Wproj"])
    U = f32(inputs["CP_U_W"])              # [D, R]
    V = f32(inputs["CP_V_W"])              # [R, D]
    CPC = f32(inputs["CP_C"])              # [a, b, r]
    CPATT = f32(inputs["CP_attention"])    # [R, 4]
    ut = np.ascontiguousarray(U.T)         # [R, D]
    cpct = np.ascontiguousarray(CPC.transpose(2, 1, 0).reshape(R, R * R))
    in_maps = []
    for c in range(NCORES):
        s = slice(c * CPB, (c + 1) * CPB)
        in_maps.append({
            "xqT": xqT, "xkT": xkT, "xvT": xvT,
            "wq_c": np.ascontiguousarray(Wq[:, s]),
            "wk_c": np.ascontiguousarray(Wk[:, s]),
            "wv_c": np.ascontiguousarray(Wv[:, s]),
            "wp_c": np.ascontiguousarray(Wp[s, :]),
            "ut": ut,
            "utc": np.ascontiguousarray(ut[:, s]),
            "vfull": V,
            "v_c": np.ascontiguousarray(V[:, s]),
            "cpct": cpct,
            "cpatt": CPATT,
        })
    return in_maps


def run(inputs, trace=False, trace_cores=None):
    nc = _get_nc()
    in_maps = _prep_in_maps(inputs)
    res = run_bass_kernel_spmd(nc, in_maps, list(range(NCORES)),
                               trace=trace, trace_cores=trace_cores)
    acc = res.results[0]["out"].astype(np.float32).copy()
    for c in range(1, NCORES):
        acc += res.results[c]["out"]
    acc += np.asarray(inputs["bproj"], dtype=np.float32)[None, :]
    return acc.reshape(B, N, D), res


def kernel(**inputs):
    out, _ = run(inputs, trace=False)
    return out
